# revision 8
# baseline (speedup 1.0000x reference)
"""MHA on 8 NeuronCores, v3: fused single-stream schedule.

Core c owns token block c = (batch c//2, seq half c%2), 1024 tokens.
v2 ran phases serially: 155us of projections before the first exp, a
344us attention phase bottlenecked on the ACT engine (~288us of exp),
then a 44us tail.  v3 starts the exp stream at ~33us and keeps ACT
saturated to the end:

  - Preamble (DMA-gated): project K (8 d-chunks, AllGather each chunk
    pairwise as it completes), V (AllGather per token quarter), then
    Q chunk 0 only.  Attention on heads 0,1 starts immediately.
  - Attention stream (qb-major: 2 query halves x 16 heads, 8 groups of
    2 key-chunks each): scores (PE) -> exp (ACT) -> PV (PE, ones-column
    in V gives the softmax denominator).  The first two units defer
    their PVs (pg backlog) until the V AllGather lands.
  - Filler: Q chunks 1-7 and the wo output projection for the first
    token half are emitted between attention groups by a build-time
    virtual-clock controller, so the PE never idles while ACT works.
  - Normalization per unit via DRAM-broadcast reciprocal; wo for the
    second token half is the only tail (~14us).

Communication: 8 pairwise K AllGathers (256KB in each) + 4 pairwise V
AllGathers (512KB in each), all overlapped.  bf16 matmuls, fp32 PSUM.
"""
import numpy as np
import ml_dtypes

import concourse.bass as bass
import concourse.bacc as bacc
import concourse.tile as tile
import concourse.mybir as mybir

N_CORES = 8
P = 128
B, S, D = 4, 2048, 1024
TOK = 1024  # my tokens
CD = D // P  # 8 chunks
QB = 512
NKC = S // P  # 16 key chunks
NU = 32  # units = 2 qb x 16 heads
F32 = mybir.dt.float32
BF16 = mybir.dt.bfloat16
EXP = mybir.ActivationFunctionType.Exp
PAIR_GROUPS = [[2 * i, 2 * i + 1] for i in range(4)]
# v_t position -> key chunk (vag quarters interleave the two cores)
KCS = [0, 1, 8, 9, 2, 3, 10, 11, 4, 5, 12, 13, 6, 7, 14, 15]
NDEFER = 2  # units whose PVs wait for the V AllGather

_CACHE = {}


def _n_excess_waits(nc):
    import json

    m = json.loads(nc.to_json_bytes())
    insts = [i for f in m["functions"] for b in f["blocks"] for i in b["instructions"]]
    return sum(
        1
        for i in insts
        if len((i.get("sync_info") or {}).get("on_wait", [])) >= 2
        and i.get("opcode") != "EventSemaphore"
    )


def _finish(nc):
    nc.compile()
    import bass_rust

    for _ in range(6):
        if _n_excess_waits(nc) == 0:
            break
        bass_rust.generate_event_semaphores(nc)
    assert _n_excess_waits(nc) == 0, "excess sync waits remain"
    nc.codegen_inst_isa_subclasses()
    return nc


def build_nc(scopes=False):
    nc = bacc.Bacc("TRN2", target_bir_lowering=False, debug=False, num_devices=N_CORES)

    xqT_d = nc.dram_tensor("xqT", [D, TOK], BF16, kind="ExternalInput").ap()
    xkT_d = nc.dram_tensor("xkT", [D, TOK], BF16, kind="ExternalInput").ap()
    xvT_d = nc.dram_tensor("xvT", [D, TOK], BF16, kind="ExternalInput").ap()
    wqkvT = nc.dram_tensor("wqkvT", [D, 3 * D], BF16, kind="ExternalInput").ap()
    woT = nc.dram_tensor("woT", [D, D], BF16, kind="ExternalInput").ap()
    out = nc.dram_tensor("out", [TOK, D], F32, kind="ExternalOutput").ap()

    kag_i = nc.dram_tensor("kag_i", [D, TOK], BF16).ap()
    kag_os = [nc.dram_tensor(f"kag_o{j}", [2, P, TOK], BF16).ap() for j in range(CD)]
    vag_i = nc.dram_tensor("vag_i", [TOK, D], BF16).ap()
    vag_os = [
        nc.dram_tensor(f"vag_o{q}", [2, TOK // 4, D], BF16).ap() for q in range(4)
    ]
    den_d = nc.dram_tensor("den_d", [NU, QB], F32).ap()
    recip_d = nc.dram_tensor("recip_d", [NU, QB], F32).ap()

    from contextlib import nullcontext

    def scope(name):
        return nc.named_scope(name) if scopes else nullcontext()

    AG_KW = dict(
        kind="AllGather", op=mybir.AluOpType.bypass, replica_groups=PAIR_GROUPS
    )

    with tile.TileContext(nc) as tc:
        # ---- persistent pools (bottom of the SBUF stack) ----------------
        # Attention-phase pools (v_t, pg backlog, norm, wo) are allocated
        # only after the K/V input pools release, so the stacked peak stays
        # under the SBUF budget.
        qp = tc.alloc_tile_pool(name="qp", bufs=1)
        kst = tc.alloc_tile_pool(name="kst", bufs=1)
        ltp = tc.alloc_tile_pool(name="ltp", bufs=1)
        evp = tc.alloc_tile_pool(name="evp", bufs=4)
        proj_ps = tc.alloc_tile_pool(name="proj_ps", bufs=2, space="PSUM")
        s_ps = tc.alloc_tile_pool(name="s_ps", bufs=2, space="PSUM")
        pv_ps = tc.alloc_tile_pool(name="pv_ps", bufs=2, space="PSUM")
        # ---- scoped pools, released LIFO: [wv,xv] [wk,xk] ---------------
        wqp = tc.alloc_tile_pool(name="wqp", bufs=1)
        xqp = tc.alloc_tile_pool(name="xqp", bufs=1)
        wvp = tc.alloc_tile_pool(name="wvp", bufs=1)
        xvp = tc.alloc_tile_pool(name="xvp", bufs=1)
        wkp = tc.alloc_tile_pool(name="wkp", bufs=1)
        xkp = tc.alloc_tile_pool(name="xkp", bufs=1)

        # ---- input DMAs in priority order -------------------------------
        with scope("load"):
            wk_t, xk_t, wv_t, xv_t, xq_t, wqc_t = [], [], [], [], [], []
            for j in range(CD):
                wk = wkp.tile([P, D], BF16, name=f"wk_{j}")
                nc.sync.dma_start(out=wk, in_=wqkvT[j * P : (j + 1) * P, D : 2 * D])
                wk_t.append(wk)
                t = xkp.tile([P, TOK], BF16, name=f"xk_{j}")
                nc.sync.dma_start(out=t, in_=xkT_d[j * P : (j + 1) * P, :])
                xk_t.append(t)
            for j in range(CD):
                wv = wvp.tile([P, D], BF16, name=f"wv_{j}")
                nc.sync.dma_start(
                    out=wv, in_=wqkvT[j * P : (j + 1) * P, 2 * D : 3 * D]
                )
                wv_t.append(wv)
                t = xvp.tile([P, TOK], BF16, name=f"xv_{j}")
                nc.sync.dma_start(out=t, in_=xvT_d[j * P : (j + 1) * P, :])
                xv_t.append(t)
            for j in range(CD):
                t = xqp.tile([P, TOK], BF16, name=f"xq_{j}")
                nc.sync.dma_start(out=t, in_=xqT_d[j * P : (j + 1) * P, :])
                xq_t.append(t)
            # wq as out-chunk column slices so chunk 0 lands first
            for i in range(CD):
                wqc = wqp.tile([P, CD, P], BF16, name=f"wqc_{i}")
                nc.sync.dma_start(
                    out=wqc,
                    in_=wqkvT[:, i * P : (i + 1) * P].rearrange(
                        "(j p) c -> p j c", p=P
                    ),
                )
                wqc_t.append(wqc)

        # ---- virtual clocks for the build-time interleave controller ---
        pe_t = [0.0]
        act_t = [0.0]

        def pe_adv(ns):
            pe_t[0] += ns

        # ---- persistent SBUF tiles --------------------------------------
        kT_s = [kst.tile([P, S], BF16, name=f"kTs_{j}") for j in range(CD)]
        qT_t = [qp.tile([P, TOK], BF16, name=f"qT_{i}") for i in range(CD)]
        lts = [ltp.tile([P, TOK], BF16, name=f"lt_{i}") for i in range(CD)]

        # ---- preamble: K projection + per-chunk AllGather ---------------
        with scope("proj_k"):
            for i in range(CD):
                for hf in range(2):
                    ps = proj_ps.tile([P, QB], F32, name="ps_p", tag="pp")
                    for j in range(CD):
                        nc.tensor.matmul(
                            ps,
                            wk_t[j][:, i * P : (i + 1) * P],
                            xk_t[j][:, hf * QB : (hf + 1) * QB],
                            start=(j == 0),
                            stop=(j == CD - 1),
                        )
                        pe_adv(QB * 0.42 + 40)
                    sb = evp.tile([P, QB], BF16, name="sb_e", tag="ev")
                    nc.vector.tensor_copy(sb, ps)
                    nc.sync.dma_start(
                        out=kag_i[i * P : (i + 1) * P, hf * QB : (hf + 1) * QB], in_=sb
                    )
                nc.gpsimd.collective_compute(
                    ins=[kag_i[i * P : (i + 1) * P, :]], outs=[kag_os[i][:]], **AG_KW
                )
                nc.sync.dma_start(out=kT_s[i][:, 0:TOK], in_=kag_os[i][0])
                nc.sync.dma_start(out=kT_s[i][:, TOK:S], in_=kag_os[i][1])
        xkp.release()
        wkp.release()

        # ---- preamble: V projection + per-quarter AllGather -------------
        with scope("proj_v"):
            for c in range(CD):
                for hf in range(2):
                    ps = proj_ps.tile([P, QB], F32, name="ps_p", tag="pp")
                    for j in range(CD):
                        nc.tensor.matmul(
                            ps,
                            xv_t[j][:, c * P : (c + 1) * P],
                            wv_t[j][:, hf * QB : (hf + 1) * QB],
                            start=(j == 0),
                            stop=(j == CD - 1),
                        )
                        pe_adv(QB * 0.42 + 40)
                    sb = evp.tile([P, QB], BF16, name="sb_e", tag="ev")
                    nc.vector.tensor_copy(sb, ps)
                    nc.sync.dma_start(
                        out=vag_i[c * P : (c + 1) * P, hf * QB : (hf + 1) * QB], in_=sb
                    )
                if c % 2 == 1:
                    q = c // 2
                    nc.gpsimd.collective_compute(
                        ins=[vag_i[q * (TOK // 4) : (q + 1) * (TOK // 4), :]],
                        outs=[vag_os[q][:]],
                        **AG_KW,
                    )
        xvp.release()
        wvp.release()
        v_ready = pe_t[0] + 12000.0

        # ---- attention-phase pools (reuse the released K/V space) -------
        vtp = tc.alloc_tile_pool(name="vtp", bufs=1)
        pgp = tc.alloc_tile_pool(name="pgp", bufs=12)
        arp = tc.alloc_tile_pool(name="arp", bufs=3)
        bcp = tc.alloc_tile_pool(name="bcp", bufs=2)
        smp = tc.alloc_tile_pool(name="smp", bufs=2)
        obp = tc.alloc_tile_pool(name="obp", bufs=2)
        wop = tc.alloc_tile_pool(name="wop", bufs=1)
        v_t = [vtp.tile([P, NKC, 65], BF16, name=f"v_{h}") for h in range(16)]

        # ---- v_t staging (DMA only; waits on the AllGathers) ------------
        with scope("vt_load"):
            for h in range(16):
                for q in range(4):
                    for half in range(2):
                        vsrc = vag_os[q][half, :, 64 * h : 64 * h + 64]
                        nc.sync.dma_start(
                            out=v_t[h][:, 4 * q + 2 * half : 4 * q + 2 * half + 2, 0:64],
                            in_=vsrc.rearrange("(kc p) d -> p kc d", p=P),
                        )
                nc.vector.memset(v_t[h][:, :, 64:65], 1.0)

        # ---- Q chunk projection (chunk 0 now, 1-7 as filler) ------------
        def emit_q_chunk(i):
            for hf in range(2):
                ps = proj_ps.tile([P, QB], F32, name="ps_p", tag="pp")
                for j in range(CD):
                    nc.tensor.matmul(
                        ps,
                        wqc_t[i][:, j, :],
                        xq_t[j][:, hf * QB : (hf + 1) * QB],
                        start=(j == 0),
                        stop=(j == CD - 1),
                    )
                    pe_adv(QB * 0.42 + 40)
                nc.vector.tensor_copy(qT_t[i][:, hf * QB : (hf + 1) * QB], ps)

        with scope("proj_q0"):
            emit_q_chunk(0)

        # ---- filler queue ----------------------------------------------
        # items: ("q", chunk) | ("wo_dma", sc) | ("wo", tchunk, half)
        filler = [("q", i) for i in range(1, CD)]
        filler += [("wo_dma", sc) for sc in range(CD)]
        fcur = [0]
        q_done = [True] + [False] * (CD - 1)
        wo_t = [None] * CD

        def run_filler_item(it):
            kind = it[0]
            if kind == "q":
                with scope(f"proj_q{it[1]}"):
                    emit_q_chunk(it[1])
                q_done[it[1]] = True
            elif kind == "wo_dma":
                sc = it[1]
                wt = wop.tile([P, D], BF16, name=f"wo_{sc}")
                nc.sync.dma_start(out=wt, in_=woT[sc * P : (sc + 1) * P, :])
                wo_t[sc] = wt
            elif kind == "wo":
                emit_wo(it[1], it[2])

        def fill_until(target_ns):
            while pe_t[0] < target_ns and fcur[0] < len(filler):
                it = filler[fcur[0]]
                fcur[0] += 1
                run_filler_item(it)

        def ensure_q(i):
            while not q_done[i]:
                it = filler[fcur[0]]
                fcur[0] += 1
                run_filler_item(it)

        def emit_wo(t_i, hf):
            with scope("wo"):
                ps = proj_ps.tile([P, QB], F32, name="ps_p", tag="pp")
                for sc in range(CD):
                    nc.tensor.matmul(
                        ps,
                        lts[sc][:, t_i * P : (t_i + 1) * P],
                        wo_t[sc][:, hf * QB : (hf + 1) * QB],
                        start=(sc == 0),
                        stop=(sc == CD - 1),
                    )
                    pe_adv(QB * 0.42 + 40)
                ob = obp.tile([P, QB], F32, name="ob", tag="ob")
                nc.vector.tensor_copy(ob, ps)
                nc.sync.dma_start(
                    out=out[t_i * P : (t_i + 1) * P, hf * QB : (hf + 1) * QB], in_=ob
                )

        # ---- attention stream -------------------------------------------
        units = [(qb, h) for qb in range(2) for h in range(16)]

        def emit_scores_group(qb, h, g):
            r = slice(64 * (h % 2), 64 * (h % 2) + 64)
            qs = slice(qb * QB, (qb + 1) * QB)
            sg = s_ps.tile([P, 2, QB], F32, name="sg", tag="sg")
            for jj in range(2):
                kc = KCS[2 * g + jj]
                nc.tensor.matmul(
                    sg[:, jj, :],
                    kT_s[h // 2][r, kc * P : (kc + 1) * P],
                    qT_t[h // 2][r, qs],
                    start=True,
                    stop=True,
                )
                pe_adv(QB * 0.42 + 40)
            pg = pgp.tile([P, 2, QB], BF16, name="pg", tag="pg")
            nc.scalar.activation(pg, sg, EXP, scale=0.125)
            act_t[0] = max(act_t[0], pe_t[0] + 200.0) + 2 * QB * 1.0 + 150.0
            return pg, act_t[0]

        def emit_pv_group(pv, h, g, pg):
            for jj in range(2):
                pos = 2 * g + jj
                nc.tensor.matmul(
                    pv,
                    v_t[h][:, pos, :],
                    pg[:, jj, :],
                    start=(pos == 0),
                    stop=(pos == NKC - 1),
                )
                pe_adv(QB * 0.42 + 40)

        def emit_norm(ui, qb, h, pv):
            with scope("norm"):
                r = slice(64 * (h % 2), 64 * (h % 2) + 64)
                qs = slice(qb * QB, (qb + 1) * QB)
                araw = arp.tile([65, QB], F32, name="araw", tag="ar")
                nc.vector.tensor_copy(araw, pv)
                nc.sync.dma_start(out=den_d[ui : ui + 1, :], in_=araw[64:65, :])
                dsq = smp.tile([64, 8], F32, name="dsq", tag="d")
                nc.sync.dma_start(
                    out=dsq,
                    in_=bass.AP(
                        tensor=den_d.tensor, offset=ui * QB, ap=[[8, 64], [1, 8]]
                    ),
                )
                rsq = smp.tile([64, 8], F32, name="rsq", tag="r")
                nc.vector.reciprocal(rsq, dsq)
                nc.sync.dma_start(
                    out=bass.AP(
                        tensor=recip_d.tensor, offset=ui * QB, ap=[[8, 64], [1, 8]]
                    ),
                    in_=rsq,
                )
                bc = bcp.tile([64, QB], F32, name="bc", tag="bc")
                nc.sync.dma_start(
                    out=bc,
                    in_=bass.AP(
                        tensor=recip_d.tensor, offset=ui * QB, ap=[[0, 64], [1, QB]]
                    ),
                )
                nc.vector.tensor_mul(lts[h // 2][r, qs], araw[0:64, :], bc)

        with scope("attn"):
            # deferred units: all scores+exp first, PVs after V arrives
            deferred = []  # (ui, qb, h, [pg]*8)
            for ui in range(NDEFER):
                qb, h = units[ui]
                ensure_q(h // 2)
                pgs = []
                for g in range(8):
                    pg, _ = emit_scores_group(qb, h, g)
                    pgs.append(pg)
                deferred.append((ui, qb, h, pgs))
            fill_until(v_ready)
            for ui, qb, h, pgs in deferred:
                pv = pv_ps.tile([65, QB], F32, name="pv", tag="pv")
                for g in range(8):
                    emit_pv_group(pv, h, g, pgs[g])
                emit_norm(ui, qb, h, pv)

            for ui in range(NDEFER, NU):
                qb, h = units[ui]
                ensure_q(h // 2)
                pv = pv_ps.tile([65, QB], F32, name="pv", tag="pv")
                prev = None  # (pg, act_done)
                for g in range(8):
                    cur = emit_scores_group(qb, h, g)
                    if prev is not None:
                        fill_until(prev[1])
                        emit_pv_group(pv, h, g - 1, prev[0])
                    prev = cur
                fill_until(prev[1])
                emit_pv_group(pv, h, 7, prev[0])
                emit_norm(ui, qb, h, pv)
                if (qb, h) == (0, 15):
                    # first token half fully normalized: wo t-chunks 0-3
                    filler.extend(
                        ("wo", t_i, hf) for t_i in range(4) for hf in range(2)
                    )

        # ---- tail: drain filler, then wo for the second token half ------
        with scope("wo_tail"):
            fill_until(float("inf"))
            for t_i in range(4, CD):
                for hf in range(2):
                    emit_wo(t_i, hf)

        # release everything in LIFO order per space
        for p in (wop, obp, smp, bcp, arp, pgp, vtp, xqp, wqp, evp, ltp, kst, qp):
            p.release()
        for p in (pv_ps, s_ps, proj_ps):
            p.release()

    return _finish(nc)


def _get_nc(scopes=False):
    key = ("nc", scopes)
    if key not in _CACHE:
        _CACHE[key] = build_nc(scopes)
    return _CACHE[key]


def make_in_maps(query, key, value, wq, wk, wv, wo):
    qf = np.asarray(query, np.float32).reshape(B * S, D)
    kf = np.asarray(key, np.float32).reshape(B * S, D)
    vf = np.asarray(value, np.float32).reshape(B * S, D)
    wqkvT = np.ascontiguousarray(
        np.concatenate([np.asarray(wq), np.asarray(wk), np.asarray(wv)], 0).T
    ).astype(ml_dtypes.bfloat16)
    woT_h = np.ascontiguousarray(np.asarray(wo).T).astype(ml_dtypes.bfloat16)
    in_maps = []
    for c in range(N_CORES):
        sl = slice(c * TOK, (c + 1) * TOK)
        in_maps.append(
            {
                "xqT": np.ascontiguousarray(qf[sl].T).astype(ml_dtypes.bfloat16),
                "xkT": np.ascontiguousarray(kf[sl].T).astype(ml_dtypes.bfloat16),
                "xvT": np.ascontiguousarray(vf[sl].T).astype(ml_dtypes.bfloat16),
                "wqkvT": wqkvT,
                "woT": woT_h,
            }
        )
    return in_maps


def assemble(results):
    blocks = [results[c]["out"] for c in range(N_CORES)]
    return np.concatenate(blocks, 0).reshape(B, S, D).astype(np.float32)


def kernel(query, key, value, mask, wq, wk, wv, wo):
    # mask is all-False in this problem: softmax without masking.
    nc = _get_nc()
    in_maps = make_in_maps(query, key, value, wq, wk, wv, wo)
    from concourse.bass_utils import run_bass_kernel_spmd

    res = run_bass_kernel_spmd(nc, in_maps, list(range(N_CORES)))
    return assemble(res.results)


# revision 10
# speedup vs baseline: 1.1603x; 1.1603x over previous
"""MHA on 8 NeuronCores, v4: fused single-stream schedule.

Core c owns token block c = (batch c//2, seq half c%2), 1024 tokens.
The exp stream on the ACT engine (~290us total) is the long pole; the
schedule starts it at ~45us and keeps it saturated:

  - DMA order: V inputs, K inputs, Q inputs (wq arrives host-swizzled
    per output chunk so chunk 0 is one contiguous load).
  - Preamble: V projection (AllGather per token quarter, kicked as
    quarters complete), K chunk 0 (+AllGather+stage), Q chunk 0, rest
    of V.  Attention on heads 0,1 starts right after.
  - Attention stream (qb-major: 2 query halves x 16 heads, 8 groups of
    2 key chunks): scores (PE) -> exp (ACT) -> PV (PE, ones-column in
    V gives the softmax denominator).  The first two units defer their
    PVs behind a pg backlog until the V AllGather lands.
  - Filler: K chunks 1-7 and Q chunks 1-7 (interleaved, in the order
    heads need them) plus the wo projection of the first token half
    are emitted between attention groups by a build-time virtual-clock
    controller in ~4-matmul micro-items, so the PE never idles while
    ACT works.  v_t staging DMAs are spread just-in-time per head.
  - Normalization per unit via DRAM-broadcast reciprocal; wo reuses the
    xq SBUF tiles; wo for the second token half is the only tail.

Communication: 8 pairwise K AllGathers (256KB in) + 4 pairwise V
AllGathers (512KB in), all overlapped.  bf16 matmuls, fp32 PSUM.
"""
import numpy as np
import ml_dtypes

import concourse.bass as bass
import concourse.bacc as bacc
import concourse.tile as tile
import concourse.mybir as mybir

N_CORES = 8
P = 128
B, S, D = 4, 2048, 1024
TOK = 1024  # my tokens
CD = D // P  # 8 chunks
QB = 512
NKC = S // P  # 16 key chunks
NU = 32  # units = 2 qb x 16 heads
F32 = mybir.dt.float32
BF16 = mybir.dt.bfloat16
EXP = mybir.ActivationFunctionType.Exp
PAIR_GROUPS = [[2 * i, 2 * i + 1] for i in range(4)]
# v_t position -> key chunk (vag quarters interleave the two cores)
KCS = [0, 1, 8, 9, 2, 3, 10, 11, 4, 5, 12, 13, 6, 7, 14, 15]
NDEFER = 2  # units whose PVs wait for the V AllGather
PG_BUFS = 12

_CACHE = {}


def _n_excess_waits(nc):
    import json

    m = json.loads(nc.to_json_bytes())
    insts = [i for f in m["functions"] for b in f["blocks"] for i in b["instructions"]]
    return sum(
        1
        for i in insts
        if len((i.get("sync_info") or {}).get("on_wait", [])) >= 2
        and i.get("opcode") != "EventSemaphore"
    )


def _finish(nc):
    nc.compile()
    import bass_rust

    for _ in range(6):
        if _n_excess_waits(nc) == 0:
            break
        bass_rust.generate_event_semaphores(nc)
    assert _n_excess_waits(nc) == 0, "excess sync waits remain"
    nc.codegen_inst_isa_subclasses()
    return nc


def build_nc(scopes=False):
    nc = bacc.Bacc("TRN2", target_bir_lowering=False, debug=False, num_devices=N_CORES)

    xqT_d = nc.dram_tensor("xqT", [D, TOK], BF16, kind="ExternalInput").ap()
    xkT_d = nc.dram_tensor("xkT", [D, TOK], BF16, kind="ExternalInput").ap()
    xvT_d = nc.dram_tensor("xvT", [D, TOK], BF16, kind="ExternalInput").ap()
    wqkvT = nc.dram_tensor("wqkvT", [D, 3 * D], BF16, kind="ExternalInput").ap()
    wq_cs = nc.dram_tensor("wq_cs", [CD, P, CD, P], BF16, kind="ExternalInput").ap()
    woT = nc.dram_tensor("woT", [D, D], BF16, kind="ExternalInput").ap()
    out = nc.dram_tensor("out", [TOK, D], F32, kind="ExternalOutput").ap()

    kag_i = nc.dram_tensor("kag_i", [D, TOK], BF16).ap()
    kag_os = [nc.dram_tensor(f"kag_o{j}", [2, P, TOK], BF16).ap() for j in range(CD)]
    vag_i = nc.dram_tensor("vag_i", [TOK, D], BF16).ap()
    vag_os = [
        nc.dram_tensor(f"vag_o{q}", [2, TOK // 4, D], BF16).ap() for q in range(4)
    ]
    den_d = nc.dram_tensor("den_d", [NU, QB], F32).ap()
    recip_d = nc.dram_tensor("recip_d", [NU, QB], F32).ap()

    from contextlib import nullcontext

    def scope(name):
        return nc.named_scope(name) if scopes else nullcontext()

    AG_KW = dict(
        kind="AllGather", op=mybir.AluOpType.bypass, replica_groups=PAIR_GROUPS
    )

    with tile.TileContext(nc) as tc:
        # ---- pools; stack order matters (LIFO release) ------------------
        qp = tc.alloc_tile_pool(name="qp", bufs=1)
        kst = tc.alloc_tile_pool(name="kst", bufs=1)
        evp = tc.alloc_tile_pool(name="evp", bufs=3)
        pgp = tc.alloc_tile_pool(name="pgp", bufs=PG_BUFS)
        proj_ps = tc.alloc_tile_pool(name="proj_ps", bufs=2, space="PSUM")
        s_ps = tc.alloc_tile_pool(name="s_ps", bufs=2, space="PSUM")
        pv_ps = tc.alloc_tile_pool(name="pv_ps", bufs=2, space="PSUM")
        wqp = tc.alloc_tile_pool(name="wqp", bufs=1)
        xqp = tc.alloc_tile_pool(name="xqp", bufs=1)
        wkp = tc.alloc_tile_pool(name="wkp", bufs=1)
        xkp = tc.alloc_tile_pool(name="xkp", bufs=1)
        wvp = tc.alloc_tile_pool(name="wvp", bufs=1)
        xvp = tc.alloc_tile_pool(name="xvp", bufs=1)

        # ---- input DMAs in priority order: V, K, Q ----------------------
        with scope("load"):
            wv_t, xv_t, wk_t, xk_t, xq_t, wqc_t = [], [], [], [], [], []
            for j in range(CD):
                wv = wvp.tile([P, D], BF16, name=f"wv_{j}")
                nc.sync.dma_start(
                    out=wv, in_=wqkvT[j * P : (j + 1) * P, 2 * D : 3 * D]
                )
                wv_t.append(wv)
                t = xvp.tile([P, TOK], BF16, name=f"xv_{j}")
                nc.sync.dma_start(out=t, in_=xvT_d[j * P : (j + 1) * P, :])
                xv_t.append(t)
            for j in range(CD):
                wk = wkp.tile([P, D], BF16, name=f"wk_{j}")
                nc.sync.dma_start(out=wk, in_=wqkvT[j * P : (j + 1) * P, D : 2 * D])
                wk_t.append(wk)
                t = xkp.tile([P, TOK], BF16, name=f"xk_{j}")
                nc.sync.dma_start(out=t, in_=xkT_d[j * P : (j + 1) * P, :])
                xk_t.append(t)
            for j in range(CD):
                t = xqp.tile([P, TOK], BF16, name=f"xq_{j}")
                nc.sync.dma_start(out=t, in_=xqT_d[j * P : (j + 1) * P, :])
                xq_t.append(t)
            for i in range(CD):
                wqc = wqp.tile([P, CD, P], BF16, name=f"wqc_{i}")
                nc.sync.dma_start(out=wqc, in_=wq_cs[i])
                wqc_t.append(wqc)

        # ---- virtual clocks ---------------------------------------------
        pe_t = [0.0]
        act_t = [0.0]

        def pe_adv(ns):
            pe_t[0] += ns

        kT_s = [kst.tile([P, S], BF16, name=f"kTs_{j}") for j in range(CD)]
        qT_t = [qp.tile([P, TOK], BF16, name=f"qT_{i}") for i in range(CD)]

        # ---- projection emitters ----------------------------------------
        psum_live = {}

        def k_part(i, hf, part):
            with scope("proj_k"):
                if part == 0:
                    psum_live[("k", i, hf)] = proj_ps.tile(
                        [P, QB], F32, name="ps_p", tag="pp"
                    )
                ps = psum_live[("k", i, hf)]
                for j in range(4 * part, 4 * part + 4):
                    nc.tensor.matmul(
                        ps,
                        wk_t[j][:, i * P : (i + 1) * P],
                        xk_t[j][:, hf * QB : (hf + 1) * QB],
                        start=(j == 0),
                        stop=(j == CD - 1),
                    )
                    pe_adv(QB * 0.42 + 40)
                if part == 1:
                    del psum_live[("k", i, hf)]
                    sb = evp.tile([P, QB], BF16, name="sb_e", tag="ev")
                    nc.vector.tensor_copy(sb, ps)
                    nc.sync.dma_start(
                        out=kag_i[i * P : (i + 1) * P, hf * QB : (hf + 1) * QB],
                        in_=sb,
                    )

        def k_ag(i):
            nc.gpsimd.collective_compute(
                ins=[kag_i[i * P : (i + 1) * P, :]], outs=[kag_os[i][:]], **AG_KW
            )
            nc.sync.dma_start(out=kT_s[i][:, 0:TOK], in_=kag_os[i][0])
            nc.sync.dma_start(out=kT_s[i][:, TOK:S], in_=kag_os[i][1])

        def v_part(c, hf, part):
            with scope("proj_v"):
                if part == 0:
                    psum_live[("v", c, hf)] = proj_ps.tile(
                        [P, QB], F32, name="ps_p", tag="pp"
                    )
                ps = psum_live[("v", c, hf)]
                for j in range(4 * part, 4 * part + 4):
                    nc.tensor.matmul(
                        ps,
                        xv_t[j][:, c * P : (c + 1) * P],
                        wv_t[j][:, hf * QB : (hf + 1) * QB],
                        start=(j == 0),
                        stop=(j == CD - 1),
                    )
                    pe_adv(QB * 0.42 + 40)
                if part == 1:
                    del psum_live[("v", c, hf)]
                    sb = evp.tile([P, QB], BF16, name="sb_e", tag="ev")
                    nc.vector.tensor_copy(sb, ps)
                    nc.sync.dma_start(
                        out=vag_i[c * P : (c + 1) * P, hf * QB : (hf + 1) * QB],
                        in_=sb,
                    )

        def q_part(i, hf, part):
            with scope("proj_q"):
                if part == 0:
                    psum_live[("q", i, hf)] = proj_ps.tile(
                        [P, QB], F32, name="ps_p", tag="pp"
                    )
                ps = psum_live[("q", i, hf)]
                for j in range(4 * part, 4 * part + 4):
                    nc.tensor.matmul(
                        ps,
                        wqc_t[i][:, j, :],
                        xq_t[j][:, hf * QB : (hf + 1) * QB],
                        start=(j == 0),
                        stop=(j == CD - 1),
                    )
                    pe_adv(QB * 0.42 + 40)
                if part == 1:
                    del psum_live[("q", i, hf)]
                    nc.vector.tensor_copy(qT_t[i][:, hf * QB : (hf + 1) * QB], ps)

        # ---- preamble: V proj (AG per quarter), K0, Q0 ------------------
        for c in range(6):
            for hf in range(2):
                v_part(c, hf, 0)
                v_part(c, hf, 1)
            if c % 2 == 1:
                q = c // 2
                nc.gpsimd.collective_compute(
                    ins=[vag_i[q * (TOK // 4) : (q + 1) * (TOK // 4), :]],
                    outs=[vag_os[q][:]],
                    **AG_KW,
                )
        for hf in range(2):
            k_part(0, hf, 0)
            k_part(0, hf, 1)
        k_ag(0)
        for hf in range(2):
            q_part(0, hf, 0)
            q_part(0, hf, 1)
        for c in range(6, CD):
            for hf in range(2):
                v_part(c, hf, 0)
                v_part(c, hf, 1)
            if c % 2 == 1:
                q = c // 2
                nc.gpsimd.collective_compute(
                    ins=[vag_i[q * (TOK // 4) : (q + 1) * (TOK // 4), :]],
                    outs=[vag_os[q][:]],
                    **AG_KW,
                )
        v_ready = pe_t[0] + 9000.0
        xvp.release()
        wvp.release()

        # ---- attention-phase pools (reuse released V-input space) -------
        vtp = tc.alloc_tile_pool(name="vtp", bufs=1)
        ltp = tc.alloc_tile_pool(name="ltp", bufs=1)
        arp = tc.alloc_tile_pool(name="arp", bufs=3)
        bcp = tc.alloc_tile_pool(name="bcp", bufs=2)
        smp = tc.alloc_tile_pool(name="smp", bufs=2)
        obp = tc.alloc_tile_pool(name="obp", bufs=2)
        v_t = [vtp.tile([P, NKC, 65], BF16, name=f"v_{h}") for h in range(16)]
        lts = [ltp.tile([P, TOK], BF16, name=f"lt_{i}") for i in range(CD)]

        vt_loaded = [False] * 16

        def vt_load(h):
            if vt_loaded[h]:
                return
            vt_loaded[h] = True
            with scope("vt_load"):
                for q in range(4):
                    for half in range(2):
                        vsrc = vag_os[q][half, :, 64 * h : 64 * h + 64]
                        nc.sync.dma_start(
                            out=v_t[h][
                                :, 4 * q + 2 * half : 4 * q + 2 * half + 2, 0:64
                            ],
                            in_=vsrc.rearrange("(kc p) d -> p kc d", p=P),
                        )
                nc.vector.memset(v_t[h][:, :, 64:65], 1.0)

        for h in range(4):
            vt_load(h)

        # ---- filler queue (order = the order heads need things) ---------
        filler = []
        for i in range(1, CD):
            for hf in range(2):
                filler.append(("k", i, hf, 0))
                filler.append(("k", i, hf, 1))
            filler.append(("kag", i))
            for hf in range(2):
                filler.append(("q", i, hf, 0))
                filler.append(("q", i, hf, 1))
        filler += [("wo_dma", sc) for sc in range(CD)]
        fcur = [0]
        kq_done = [1] + [0] * (CD - 1)  # counts: 1 == chunk pair fully emitted
        wo_t = xq_t  # wo reuses the xq SBUF tiles (loaded after Q reads them)

        def run_filler_item(it):
            kind = it[0]
            if kind == "k":
                k_part(it[1], it[2], it[3])
            elif kind == "kag":
                k_ag(it[1])
            elif kind == "q":
                q_part(it[1], it[2], it[3])
                if it[2] == 1 and it[3] == 1:
                    kq_done[it[1]] = 1
            elif kind == "wo_dma":
                sc = it[1]
                nc.sync.dma_start(out=wo_t[sc], in_=woT[sc * P : (sc + 1) * P, :])
            elif kind == "wo":
                wo_part(it[1], it[2], it[3])

        def fill_until(target_ns):
            while pe_t[0] < target_ns and fcur[0] < len(filler):
                it = filler[fcur[0]]
                fcur[0] += 1
                run_filler_item(it)

        def ensure_kq(i):
            while not kq_done[i]:
                it = filler[fcur[0]]
                fcur[0] += 1
                run_filler_item(it)

        def wo_part(t_i, hf, part):
            with scope("wo"):
                if part == 0:
                    psum_live[("wo", t_i, hf)] = proj_ps.tile(
                        [P, QB], F32, name="ps_p", tag="pp"
                    )
                ps = psum_live[("wo", t_i, hf)]
                for sc in range(4 * part, 4 * part + 4):
                    nc.tensor.matmul(
                        ps,
                        lts[sc][:, t_i * P : (t_i + 1) * P],
                        wo_t[sc][:, hf * QB : (hf + 1) * QB],
                        start=(sc == 0),
                        stop=(sc == CD - 1),
                    )
                    pe_adv(QB * 0.42 + 40)
                if part == 1:
                    del psum_live[("wo", t_i, hf)]
                    ob = obp.tile([P, QB], F32, name="ob", tag="ob")
                    nc.vector.tensor_copy(ob, ps)
                    nc.sync.dma_start(
                        out=out[t_i * P : (t_i + 1) * P, hf * QB : (hf + 1) * QB],
                        in_=ob,
                    )

        # ---- attention stream -------------------------------------------
        units = [(qb, h) for qb in range(2) for h in range(16)]

        def emit_scores_group(qb, h, g):
            r = slice(64 * (h % 2), 64 * (h % 2) + 64)
            qs = slice(qb * QB, (qb + 1) * QB)
            sg = s_ps.tile([P, 2, QB], F32, name="sg", tag="sg")
            for jj in range(2):
                kc = KCS[2 * g + jj]
                nc.tensor.matmul(
                    sg[:, jj, :],
                    kT_s[h // 2][r, kc * P : (kc + 1) * P],
                    qT_t[h // 2][r, qs],
                    start=True,
                    stop=True,
                )
                pe_adv(QB * 0.42 + 40)
            pg = pgp.tile([P, 2, QB], BF16, name="pg", tag="pg")
            nc.scalar.activation(pg, sg, EXP, scale=0.125)
            act_t[0] = max(act_t[0], pe_t[0] + 200.0) + 2 * QB * 1.0 + 150.0
            return pg, act_t[0]

        def emit_pv_group(pv, h, g, pg):
            for jj in range(2):
                pos = 2 * g + jj
                nc.tensor.matmul(
                    pv,
                    v_t[h][:, pos, :],
                    pg[:, jj, :],
                    start=(pos == 0),
                    stop=(pos == NKC - 1),
                )
                pe_adv(QB * 0.42 + 40)

        def emit_norm(ui, qb, h, pv):
            with scope("norm"):
                r = slice(64 * (h % 2), 64 * (h % 2) + 64)
                qs = slice(qb * QB, (qb + 1) * QB)
                araw = arp.tile([65, QB], F32, name="araw", tag="ar")
                nc.vector.tensor_copy(araw, pv)
                nc.sync.dma_start(out=den_d[ui : ui + 1, :], in_=araw[64:65, :])
                dsq = smp.tile([64, 8], F32, name="dsq", tag="d")
                nc.sync.dma_start(
                    out=dsq,
                    in_=bass.AP(
                        tensor=den_d.tensor, offset=ui * QB, ap=[[8, 64], [1, 8]]
                    ),
                )
                rsq = smp.tile([64, 8], F32, name="rsq", tag="r")
                nc.vector.reciprocal(rsq, dsq)
                nc.sync.dma_start(
                    out=bass.AP(
                        tensor=recip_d.tensor, offset=ui * QB, ap=[[8, 64], [1, 8]]
                    ),
                    in_=rsq,
                )
                bc = bcp.tile([64, QB], F32, name="bc", tag="bc")
                nc.sync.dma_start(
                    out=bc,
                    in_=bass.AP(
                        tensor=recip_d.tensor, offset=ui * QB, ap=[[0, 64], [1, QB]]
                    ),
                )
                nc.vector.tensor_mul(lts[h // 2][r, qs], araw[0:64, :], bc)

        with scope("attn"):
            deferred = []
            for ui in range(NDEFER):
                qb, h = units[ui]
                pgs = []
                for g in range(8):
                    pg, ad = emit_scores_group(qb, h, g)
                    pgs.append(pg)
                    fill_until(ad - 2000.0)
                deferred.append((ui, qb, h, pgs))
            fill_until(v_ready)
            for ui, qb, h, pgs in deferred:
                pv = pv_ps.tile([65, QB], F32, name="pv", tag="pv")
                for g in range(8):
                    emit_pv_group(pv, h, g, pgs[g])
                emit_norm(ui, qb, h, pv)

            for ui in range(NDEFER, NU):
                qb, h = units[ui]
                ensure_kq(h // 2)
                if qb == 0 and h + 2 < 16:
                    vt_load(h + 2)
                pv = pv_ps.tile([65, QB], F32, name="pv", tag="pv")
                prev = None  # (pg, act_done)
                for g in range(8):
                    cur = emit_scores_group(qb, h, g)
                    if prev is not None:
                        fill_until(prev[1])
                        emit_pv_group(pv, h, g - 1, prev[0])
                    prev = cur
                fill_until(prev[1])
                emit_pv_group(pv, h, 7, prev[0])
                emit_norm(ui, qb, h, pv)
                if (qb, h) == (0, 15):
                    filler.extend(
                        ("wo", t_i, hf, part)
                        for t_i in range(4)
                        for hf in range(2)
                        for part in range(2)
                    )

        # ---- tail: drain filler, then wo for the second token half ------
        with scope("wo_tail"):
            fill_until(float("inf"))
            for t_i in range(4, CD):
                for hf in range(2):
                    wo_part(t_i, hf, 0)
                    wo_part(t_i, hf, 1)

        for p in (obp, smp, bcp, arp, ltp, vtp, xkp, wkp, xqp, wqp, pgp, evp, kst, qp):
            p.release()
        for p in (pv_ps, s_ps, proj_ps):
            p.release()

    return _finish(nc)


def _get_nc(scopes=False):
    key = ("nc", scopes)
    if key not in _CACHE:
        _CACHE[key] = build_nc(scopes)
    return _CACHE[key]


def make_in_maps(query, key, value, wq, wk, wv, wo):
    qf = np.asarray(query, np.float32).reshape(B * S, D)
    kf = np.asarray(key, np.float32).reshape(B * S, D)
    vf = np.asarray(value, np.float32).reshape(B * S, D)
    wqkvT = np.ascontiguousarray(
        np.concatenate([np.asarray(wq), np.asarray(wk), np.asarray(wv)], 0).T
    ).astype(ml_dtypes.bfloat16)
    wqT = np.ascontiguousarray(np.asarray(wq, np.float32).T)
    wq_cs = np.ascontiguousarray(
        wqT.reshape(CD, P, CD, P).transpose(2, 1, 0, 3)
    ).astype(ml_dtypes.bfloat16)
    woT_h = np.ascontiguousarray(np.asarray(wo).T).astype(ml_dtypes.bfloat16)
    in_maps = []
    for c in range(N_CORES):
        sl = slice(c * TOK, (c + 1) * TOK)
        in_maps.append(
            {
                "xqT": np.ascontiguousarray(qf[sl].T).astype(ml_dtypes.bfloat16),
                "xkT": np.ascontiguousarray(kf[sl].T).astype(ml_dtypes.bfloat16),
                "xvT": np.ascontiguousarray(vf[sl].T).astype(ml_dtypes.bfloat16),
                "wqkvT": wqkvT,
                "wq_cs": wq_cs,
                "woT": woT_h,
            }
        )
    return in_maps


def assemble(results):
    blocks = [results[c]["out"] for c in range(N_CORES)]
    return np.concatenate(blocks, 0).reshape(B, S, D).astype(np.float32)


def kernel(query, key, value, mask, wq, wk, wv, wo):
    # mask is all-False in this problem: softmax without masking.
    nc = _get_nc()
    in_maps = make_in_maps(query, key, value, wq, wk, wv, wo)
    from concourse.bass_utils import run_bass_kernel_spmd

    res = run_bass_kernel_spmd(nc, in_maps, list(range(N_CORES)))
    return assemble(res.results)
